# revision 1
# baseline (speedup 1.0000x reference)
"""Trainium2 Bass kernel for nn_Decoder (attention + LSTM decoder + vocab proj).

Sharding across the 8 NeuronCores of one trn2 chip (SPMD, one program):
 - attention block replicated (tiny);
 - embedding lookup via dma_gather (transpose mode), replicated;
 - LSTM gates sharded 8-way: core k owns hidden units [128k,128k+128) and
   computes their i/f/g/o gate slices; per-step h_k.T [128,48] bf16 slices
   are exchanged with an AllGather collective into every core's hs.T buffer;
 - hs.T doubles as the stationary operand of the final vocab projection,
   sharded over V (4000 cols/core), interleaved with the scan.

Compute dtype bf16 on TensorE, f32 state + PSUM accumulation.
"""
import numpy as np
import ml_dtypes

from concourse import bacc, bass, tile, mybir
from concourse.bass_utils import run_bass_kernel_spmd

F32 = mybir.dt.float32
BF16 = mybir.dt.bfloat16
I16 = mybir.dt.int16
AF = mybir.ActivationFunctionType
ALU = mybir.AluOpType
BF = ml_dtypes.bfloat16

T, B, S, D, E, V = 64, 48, 64, 1024, 300, 32000
NC = 8               # cores
TB = T * B           # 3072 tokens
KT = D // 128        # 8 k-tiles over hidden dim
VS = V // NC         # 4000 vocab per core
EP = 384             # padded embedding dim (768B rows for gather transpose)
ET = EP // 128       # 3 emb k-tiles
MT = TB // 128       # 24 M-tiles of final matmul
NCH = 8              # vocab chunks per core
VCH = VS // NCH      # 500 cols = 1 PSUM bank


def _bf(x):
    return np.ascontiguousarray(np.asarray(x, np.float32).astype(BF))


def _f32(x):
    return np.ascontiguousarray(np.asarray(x, np.float32))


def _lhsT_rows(matT, nk, nm):
    """matT [K, M] -> partition-major tile strip [128, nk*nm*128]:
    row p holds lhsT tiles (kk, m) laid out as [(kk*nm+m)*128 + q]."""
    K, M = matT.shape
    assert K == nk * 128 and M == nm * 128
    t = matT.reshape(nk, 128, nm, 128).transpose(1, 0, 2, 3)
    return np.ascontiguousarray(t.reshape(128, nk * nm * 128))


def prep_in_maps(targ, encoder_out, emb, attin_w, attin_b, attout_w, attout_b,
                 gen_w, gen_b, hidden, w_ih, w_hh, b_ih, b_hh):
    ids = np.asarray(targ).reshape(TB)  # t*48+b order
    embedded = _f32(emb)[ids]                      # [3072, 300] host gather
    embTf = np.zeros((EP, TB), np.float32)
    embTf[:E, :] = embedded.T
    embT_in = _bf(np.ascontiguousarray(
        embTf.reshape(ET, 128, TB).transpose(1, 0, 2)).reshape(128, ET * TB))

    encT = np.ascontiguousarray(
        _f32(encoder_out).transpose(2, 0, 1).reshape(KT, 128, S * B))

    hidT = np.ascontiguousarray(
        _f32(hidden).T.reshape(KT, 128, B).transpose(1, 0, 2)
    ).reshape(128, KT * B)

    attin_sb = _lhsT_rows(_f32(attin_w).T, KT, KT)        # [128, 64*128]
    attout_sb = _lhsT_rows(_f32(attout_w).T, 2 * KT, KT)   # [128,128*128]
    attin_bT = _f32(attin_b).reshape(KT, 128).T.copy()
    attout_bT = _f32(attout_b).reshape(KT, 128).T.copy()

    w_ih = _f32(w_ih)
    w_hh = _f32(w_hh)
    bsum = _f32(b_ih) + _f32(b_hh)

    ident = np.zeros((128, 128), np.float32)
    np.fill_diagonal(ident, 1.0)
    base = {
        "encT": encT, "hidT": hidT, "ident_bf": _bf(ident),
        "attin_w_sb": attin_sb, "attout_w_sb": attout_sb,
        "attin_bT": attin_bT, "attout_bT": attout_bT,
        "embT_in": embT_in,
    }

    in_maps = []
    for k in range(NC):
        rows = np.concatenate(
            [np.arange(g * D + k * 128, g * D + (k + 1) * 128)
             for g in range(4)])              # [512] i/f/g/o rows of core k
        wsel = w_ih[rows]
        wctx = wsel[:, :D]
        wemb = np.zeros((512, EP), np.float32)
        wemb[:, :E] = wsel[:, D:]
        whh = w_hh[rows]
        m = dict(base)
        m.update({
            "wctx_sb": _bf(_lhsT_rows(wctx.T, KT, 4)),   # [128, 32*128]
            "wemb_sb": _bf(_lhsT_rows(wemb.T, ET, 4)),   # [128, 12*128]
            "whh_sb": _bf(_lhsT_rows(whh.T, KT, 4)),     # [128, 32*128]
            "bias_kT": bsum[rows].reshape(4, 128).T.copy(),
            "gen_sb": _bf(np.ascontiguousarray(
                _f32(gen_w)[k * VS:(k + 1) * VS].T.reshape(KT, 128, VS)
                .transpose(1, 0, 2)).reshape(128, KT * VS)),
            "genb_bc": np.ascontiguousarray(np.broadcast_to(
                _f32(gen_b)[k * VS:(k + 1) * VS], (128, VS))),
        })
        in_maps.append(m)
    return in_maps


def build_kernel():
    nc = bacc.Bacc(None, num_devices=NC, target_bir_lowering=False,
                   debug=False)
    dp = nc.declare_dram_parameter
    encT = dp("encT", [KT, 128, S * B], F32, isOutput=False)
    hidT = dp("hidT", [128, KT * B], F32, isOutput=False)
    attin_w_d = dp("attin_w_sb", [128, KT * KT * 128], F32, isOutput=False)
    attout_w_d = dp("attout_w_sb", [128, 2 * KT * KT * 128], F32,
                    isOutput=False)
    attin_bT = dp("attin_bT", [128, KT], F32, isOutput=False)
    attout_bT = dp("attout_bT", [128, KT], F32, isOutput=False)
    embT_d = dp("embT_in", [128, ET * TB], BF16, isOutput=False)
    ident_d = dp("ident_bf", [128, 128], BF16, isOutput=False)
    wctx_d = dp("wctx_sb", [128, KT * 4 * 128], BF16, isOutput=False)
    wemb_d = dp("wemb_sb", [128, ET * 4 * 128], BF16, isOutput=False)
    whh_d = dp("whh_sb", [128, KT * 4 * 128], BF16, isOutput=False)
    bias_kT = dp("bias_kT", [128, 4], F32, isOutput=False)
    gen_d = dp("gen_sb", [128, KT * VS], BF16, isOutput=False)
    genb_d = dp("genb_bc", [128, VS], F32, isOutput=False)
    out = dp("out", [TB, VS], F32, isOutput=True)

    ag_outs = [
        nc.dram_tensor(f"ag{t}", [NC * 128, B], BF16, addr_space="Shared")
        for t in range(T)
    ]

    with tile.TileContext(nc) as tc:
        with (
            tc.tile_pool(name="live", bufs=1) as live,
        ):
            # ---- long-lived SBUF ----
            hsT = live.tile([128, KT, TB], BF16)          # gathered h.T
            xgT = live.tile([128, 4, TB], BF16)           # gate preacts
            ctxT = live.tile([128, KT * B], BF16)         # context.T
            cT = live.tile([128, B], F32)
            ident_sb = live.tile([128, 128], BF16)
            nc.sync.dma_start(ident_sb[:], ident_d[:])

            # =========== Phase A: attention (all f32) ===========
            with (
                tc.tile_pool(name="pa", bufs=1) as pa,
                tc.tile_pool(name="psa", bufs=1, space="PSUM") as psa,
            ):
                hid_sb = pa.tile([128, KT * B], F32)
                nc.sync.dma_start(hid_sb[:], hidT[:])
                ainb_sb = pa.tile([128, KT], F32)
                nc.sync.dma_start(ainb_sb[:], attin_bT[:])
                aoutb_sb = pa.tile([128, KT], F32)
                nc.sync.dma_start(aoutb_sb[:], attout_bT[:])
                htT = pa.tile([128, KT * B], F32)
                ones32 = pa.tile([128, 1], F32)
                nc.vector.memset(ones32[:], 1.0)
                svb = pa.tile([128, KT * B], F32)
                a_bc = pa.tile([128, S * B], F32)

                with (
                    tc.tile_pool(name="pa1", bufs=1) as pa1,
                    tc.tile_pool(name="pa2", bufs=2) as pa2,
                ):
                    attin_sb = pa1.tile([128, KT * KT * 128], F32)
                    nc.sync.dma_start(attin_sb[:], attin_w_d[:])
                    for j in range(KT):
                        ps_ht = psa.tile([128, B], F32, name="ps_ht",
                                         tag="ps_ht")
                        for kk in range(KT):
                            nc.tensor.matmul(
                                ps_ht[:],
                                attin_sb[:, (kk * KT + j) * 128:
                                         (kk * KT + j + 1) * 128],
                                hid_sb[:, kk * B:(kk + 1) * B],
                                start=(kk == 0), stop=(kk == KT - 1))
                        nc.scalar.activation(htT[:, j * B:(j + 1) * B],
                                             ps_ht[:], AF.Identity,
                                             bias=ainb_sb[:, j:j + 1])

                    # sc[s,b] = sum_d enc*ht -> PSUM [1, 3072] via ones-matmul
                    ps_sc = psa.tile([1, S * B], F32, name="ps_sc",
                                     tag="ps_sc")
                    for j in range(KT):
                        enc_sb = pa2.tile([128, S * B], F32, name="enc_sb",
                                          tag="enc")
                        nc.sync.dma_start(enc_sb[:], encT[j])
                        prod = pa2.tile([128, S * B], F32, name="prod",
                                        tag="prod")
                        nc.vector.tensor_tensor(
                            prod[:].rearrange("p (s b) -> p s b", s=S),
                            enc_sb[:].rearrange("p (s b) -> p s b", s=S),
                            htT[:, j * B:(j + 1) * B].unsqueeze(1)
                               .broadcast_to([128, S, B]),
                            ALU.mult)
                        for c in range(6):
                            nc.tensor.matmul(
                                ps_sc[:, c * 512:(c + 1) * 512],
                                ones32[:],
                                prod[:, c * 512:(c + 1) * 512],
                                start=(j == 0), stop=(j == KT - 1))

                    # log_softmax over b, on partition-0 row [1, 3072]
                    sc_row = pa1.tile([1, S * B], F32)
                    nc.vector.tensor_copy(sc_row[:], ps_sc[:])
                    scv = sc_row[:].rearrange("o (s b) -> o s b", s=S)
                    mx = pa1.tile([1, S], F32)
                    nc.vector.tensor_reduce(mx[:], scv, mybir.AxisListType.X,
                                            ALU.max)
                    mxb = mx[:].unsqueeze(2).broadcast_to([1, S, B])
                    exr = pa1.tile([1, S * B], F32)
                    nc.vector.tensor_tensor(
                        exr[:].rearrange("o (s b) -> o s b", s=S), scv, mxb,
                        ALU.subtract)
                    nc.scalar.activation(exr[:], exr[:], AF.Exp)
                    sume = pa1.tile([1, S], F32)
                    nc.vector.tensor_reduce(
                        sume[:], exr[:].rearrange("o (s b) -> o s b", s=S),
                        mybir.AxisListType.X, ALU.add)
                    lnz = pa1.tile([1, S], F32)
                    nc.scalar.activation(lnz[:], sume[:], AF.Ln)
                    tot = pa1.tile([1, S], F32)
                    nc.vector.tensor_tensor(tot[:], lnz[:], mx[:], ALU.add)
                    a_row = pa1.tile([1, S * B], F32)
                    nc.vector.tensor_tensor(
                        a_row[:].rearrange("o (s b) -> o s b", s=S), scv,
                        tot[:].unsqueeze(2).broadcast_to([1, S, B]),
                        ALU.subtract)
                    # broadcast a_row to all partitions via K=1 ones-matmul
                    ones128 = pa1.tile([1, 128], F32)
                    nc.vector.memset(ones128[:], 1.0)
                    for c in range(6):
                        ps_ab = psa.tile([128, 512], F32, name="ps_ab",
                                         tag="ps_ab")
                        nc.tensor.matmul(ps_ab[:], ones128[:],
                                         a_row[0:1, c * 512:(c + 1) * 512],
                                         start=True, stop=True)
                        nc.vector.tensor_copy(a_bc[:, c * 512:(c + 1) * 512],
                                              ps_ab[:])

                    # s_vec.T [128, 8*48]
                    for j in range(KT):
                        enc_sb = pa2.tile([128, S * B], F32, name="enc_sb",
                                          tag="enc")
                        nc.sync.dma_start(enc_sb[:], encT[j])
                        prod2 = pa2.tile([128, S * B], F32, name="prod2",
                                         tag="prod")
                        nc.vector.tensor_tensor(prod2[:], enc_sb[:], a_bc[:],
                                                ALU.mult)
                        sv_f = pa2.tile([128, B], F32, name="sv_f",
                                        tag="sv_f")
                        nc.vector.tensor_reduce(
                            sv_f[:],
                            prod2[:].rearrange("p (s b) -> p b s", s=S),
                            mybir.AxisListType.X, ALU.add)
                        nc.vector.tensor_copy(svb[:, j * B:(j + 1) * B],
                                              sv_f[:])

                with tc.tile_pool(name="pa3", bufs=1) as pa3:
                    attout_sb = pa3.tile([128, 2 * KT * KT * 128], F32)
                    nc.sync.dma_start(attout_sb[:], attout_w_d[:])
                    for m in range(KT):
                        ps_ctx = psa.tile([128, B], F32, name="ps_ctx",
                                          tag="ps_ht")
                        for kk in range(2 * KT):
                            rhs = (svb[:, kk * B:(kk + 1) * B] if kk < KT
                                   else hid_sb[:, (kk - KT) * B:
                                               (kk - KT + 1) * B])
                            nc.tensor.matmul(
                                ps_ctx[:],
                                attout_sb[:, (kk * KT + m) * 128:
                                          (kk * KT + m + 1) * 128],
                                rhs, start=(kk == 0), stop=(kk == 2 * KT - 1))
                        nc.scalar.activation(
                            ctxT[:, m * B:(m + 1) * B], ps_ctx[:], AF.Tanh,
                            bias=aoutb_sb[:, m:m + 1])

            # =========== Phase B: embedding + gate preacts ===========
            with (
                tc.tile_pool(name="pb", bufs=1) as pb,
                tc.tile_pool(name="psb", bufs=2, space="PSUM") as psb,
            ):
                embT = pb.tile([128, ET, TB], BF16)
                nc.sync.dma_start(
                    embT[:].rearrange("p a n -> p (a n)"), embT_d[:])

                wctx_sb = pb.tile([128, KT * 4 * 128], BF16)
                nc.sync.dma_start(wctx_sb[:], wctx_d[:])
                wemb_sb = pb.tile([128, ET * 4 * 128], BF16)
                nc.sync.dma_start(wemb_sb[:], wemb_d[:])
                bias_sb = pb.tile([128, 4], F32)
                nc.sync.dma_start(bias_sb[:], bias_kT[:])

                # ctx gate contribution (constant over t) + bias
                ctxg = pb.tile([128, 4, B], F32)
                for g in range(4):
                    ps_cg = psb.tile([128, B], F32, name="ps_cg", tag="ps_cg")
                    for kk in range(KT):
                        nc.tensor.matmul(
                            ps_cg[:],
                            wctx_sb[:, (kk * 4 + g) * 128:
                                    (kk * 4 + g + 1) * 128],
                            ctxT[:, kk * B:(kk + 1) * B],
                            start=(kk == 0), stop=(kk == KT - 1))
                    nc.scalar.activation(ctxg[:, g, :], ps_cg[:], AF.Identity,
                                         bias=bias_sb[:, g:g + 1])

                # xgT = emb part + ctxg broadcast; chunks of 8 timesteps
                for g in range(4):
                    for c in range(8):
                        ps_xg = psb.tile([128, 8 * B], F32, name="ps_xg",
                                         tag="ps_xg")
                        for kk in range(ET):
                            nc.tensor.matmul(
                                ps_xg[:],
                                wemb_sb[:, (kk * 4 + g) * 128:
                                        (kk * 4 + g + 1) * 128],
                                embT[:, kk, c * 8 * B:(c + 1) * 8 * B],
                                start=(kk == 0), stop=(kk == ET - 1))
                        nc.vector.tensor_tensor(
                            xgT[:, g, c * 8 * B:(c + 1) * 8 * B]
                            .rearrange("p (t b) -> p t b", b=B),
                            ps_xg[:].rearrange("p (t b) -> p t b", b=B),
                            ctxg[:, g, :].unsqueeze(1)
                                         .broadcast_to([128, 8, B]),
                            ALU.add)

            # =========== Phase C: scan + vocab projection ===========
            with (
                tc.tile_pool(name="pc", bufs=1) as pc,
                tc.tile_pool(name="pc3", bufs=3) as pc3,
                tc.tile_pool(name="pdram", bufs=3, space="DRAM") as pdram,
                tc.tile_pool(name="psc", bufs=1, space="PSUM") as psc,
                tc.tile_pool(name="psf", bufs=2, space="PSUM") as psf,
            ):
                gen_sb = pc.tile([128, KT * VS], BF16)
                nc.sync.dma_start(gen_sb[:], gen_d[:])
                whh_sb = pc.tile([128, KT * 4 * 128], BF16)
                nc.sync.dma_start(whh_sb[:], whh_d[:])
                genb_sb = pc.tile([128, VS], F32)
                nc.sync.dma_start(genb_sb[:], genb_d[:])

                def emit_chunk(m, n):
                    ps_f = psf.tile([128, VCH], F32, name="ps_f", tag="ps_f")
                    for kk in range(KT):
                        nc.tensor.matmul(
                            ps_f[:],
                            hsT[:, kk, m * 128:(m + 1) * 128],
                            gen_sb[:, kk * VS + n * VCH:
                                   kk * VS + (n + 1) * VCH],
                            start=(kk == 0), stop=(kk == KT - 1))
                    o_sb = pc3.tile([128, VCH], F32, name="o_sb", tag="o_sb")
                    nc.vector.tensor_tensor(
                        o_sb[:], ps_f[:],
                        genb_sb[:, n * VCH:(n + 1) * VCH], ALU.add)
                    nc.sync.dma_start(
                        out[m * 128:(m + 1) * 128,
                            n * VCH:(n + 1) * VCH], o_sb[:])

                next_m = 0
                next_n = 0
                jobs_done = 0
                for t in range(T):
                    ps_g = []
                    for g in range(4):
                        ps = psc.tile([128, B], F32, name=f"ps_g{g}",
                                      tag=f"ps_g{g}")
                        nc.tensor.matmul(
                            ps[:], ident_sb[:],
                            xgT[:, g, t * B:(t + 1) * B],
                            start=True, stop=(t == 0))
                        if t > 0:
                            for kk in range(KT):
                                nc.tensor.matmul(
                                    ps[:],
                                    whh_sb[:, (kk * 4 + g) * 128:
                                           (kk * 4 + g + 1) * 128],
                                    hsT[:, kk, (t - 1) * B:t * B],
                                    start=False, stop=(kk == KT - 1))
                        ps_g.append(ps)

                    si = pc3.tile([128, B], F32, name="si", tag="si")
                    sf = pc3.tile([128, B], F32, name="sf", tag="sf")
                    tg = pc3.tile([128, B], F32, name="tg", tag="tg")
                    so = pc3.tile([128, B], F32, name="so", tag="so")
                    nc.scalar.activation(si[:], ps_g[0][:], AF.Sigmoid)
                    nc.scalar.activation(sf[:], ps_g[1][:], AF.Sigmoid)
                    nc.scalar.activation(tg[:], ps_g[2][:], AF.Tanh)
                    nc.scalar.activation(so[:], ps_g[3][:], AF.Sigmoid)

                    m2 = pc3.tile([128, B], F32, name="m2", tag="m2")
                    nc.vector.tensor_tensor(m2[:], si[:], tg[:], ALU.mult)
                    if t == 0:
                        nc.vector.tensor_copy(cT[:], m2[:])
                    else:
                        m1 = pc3.tile([128, B], F32, name="m1", tag="m1")
                        nc.vector.tensor_tensor(m1[:], sf[:], cT[:], ALU.mult)
                        nc.vector.tensor_tensor(cT[:], m1[:], m2[:], ALU.add)
                    tc_ = pc3.tile([128, B], F32, name="tc_", tag="tc_")
                    nc.scalar.activation(tc_[:], cT[:], AF.Tanh)
                    h_bf = pc3.tile([128, B], BF16, name="h_bf", tag="h_bf")
                    nc.vector.tensor_tensor(h_bf[:], so[:], tc_[:], ALU.mult)

                    hb_d = pdram.tile([128, B], BF16, name="hb_d", tag="hb_d")
                    nc.sync.dma_start(hb_d[:], h_bf[:])
                    nc.gpsimd.collective_compute(
                        "AllGather", ALU.bypass,
                        replica_groups=[list(range(NC))],
                        ins=[hb_d.opt()],
                        outs=[ag_outs[t][:]],
                    )
                    nc.sync.dma_start(
                        hsT[:, :, t * B:(t + 1) * B],
                        ag_outs[t][:].rearrange("(j p) b -> p j b", p=128))

                    # spread vocab-projection chunks into the AG gaps:
                    # emit up to 4 ready chunks per step
                    avail_m = ((t + 1) * B) // 128   # M-tiles fully gathered
                    emitted = 0
                    while emitted < 4 and next_m < avail_m:
                        emit_chunk(next_m, next_n)
                        next_n += 1
                        emitted += 1
                        if next_n == NCH:
                            next_n = 0
                            next_m += 1
                while next_m < MT:
                    emit_chunk(next_m, next_n)
                    next_n += 1
                    if next_n == NCH:
                        next_n = 0
                        next_m += 1
    nc.compile()
    return nc


_NC_CACHE = {}


def get_nc():
    if "nc" not in _NC_CACHE:
        _NC_CACHE["nc"] = build_kernel()
    return _NC_CACHE["nc"]


def kernel(**inputs):
    in_maps = prep_in_maps(**inputs)
    nc = get_nc()
    res = run_bass_kernel_spmd(nc, in_maps, core_ids=list(range(NC)))
    shards = [np.asarray(res.results[k]["out"]).reshape(T, B, VS)
              for k in range(NC)]
    return np.concatenate(shards, axis=2)


if __name__ == "__main__":
    print("building...")
    get_nc()
    print("built ok")



# revision 19
# speedup vs baseline: 1.0570x; 1.0570x over previous
"""Trainium2 Bass kernel for nn_Decoder (attention + LSTM decoder + vocab proj).

Sharding across the 8 NeuronCores of one trn2 chip (SPMD, one program):
 - attention block replicated (fp16 score path: the legacy log-softmax
   weights enter linearly with |a|~45, so bf16's absolute rounding there
   is too coarse; fp16 keeps the end-to-end error under 1e-2);
 - embedding gather done on host, gate preacts precomputed for all t;
 - LSTM gates sharded 8-way: core k owns hidden units [128k,128k+128) and
   computes their i/f/o/g gate slices; per-step h_k.T [128,48] bf16 slices
   are exchanged with an AllGather collective into every core's hs.T buffer;
 - hs.T doubles as the stationary operand of the final vocab projection,
   sharded over V (4000 cols/core); chunks are emitted AFTER each step's
   AllGather trigger and only for rows gathered in PRIOR steps, so the
   TensorE streams vocab matmuls during the collective's latency window.

Compute dtype bf16 on TensorE, f32 state + PSUM accumulation.
"""
import numpy as np
import ml_dtypes

from concourse import bacc, bass, tile, mybir
from concourse.bass_utils import run_bass_kernel_spmd

F32 = mybir.dt.float32
F32R = mybir.dt.float32r
F16 = mybir.dt.float16
BF16 = mybir.dt.bfloat16
AF = mybir.ActivationFunctionType
ALU = mybir.AluOpType
BF = ml_dtypes.bfloat16

T, B, S, D, E, V = 64, 48, 64, 1024, 300, 32000
NC = 8               # cores
TB = T * B           # 3072 tokens
KT = D // 128        # 8 k-tiles over hidden dim
VS = V // NC         # 4000 vocab per core
EP = 384             # padded embedding dim (768B rows for gather transpose)
ET = EP // 128       # 3 emb k-tiles
MT = TB // 128       # 24 M-tiles of final matmul
NCH = 8              # vocab chunks per core
VCH = VS // NCH      # 500 cols = 1 PSUM bank
GATE_ORDER = (0, 1, 3, 2)   # pytorch i,f,g,o -> i,f,o,g (sigmoid group first)


def _bf(x):
    return np.ascontiguousarray(np.asarray(x, np.float32).astype(BF))


def _f16(x):
    return np.ascontiguousarray(np.asarray(x, np.float32).astype(np.float16))


def _f32(x):
    return np.ascontiguousarray(np.asarray(x, np.float32))


def _lhsT_rows(matT, nk, nm):
    """matT [K, M] -> partition-major tile strip [128, nk*nm*128]:
    row p holds lhsT tiles (kk, m) laid out as [(kk*nm+m)*128 + q]."""
    K, M = matT.shape
    assert K == nk * 128 and M == nm * 128
    t = matT.reshape(nk, 128, nm, 128).transpose(1, 0, 2, 3)
    return np.ascontiguousarray(t.reshape(128, nk * nm * 128))


def prep_in_maps(targ, encoder_out, emb, attin_w, attin_b, attout_w, attout_b,
                 gen_w, gen_b, hidden, w_ih, w_hh, b_ih, b_hh):
    ids = np.asarray(targ).reshape(TB)  # t*48+b order
    embedded = _f32(emb)[ids]                      # [3072, 300] host gather
    embTf = np.zeros((EP, TB), np.float32)
    embTf[:E, :] = embedded.T
    embT_in = _bf(np.ascontiguousarray(
        embTf.reshape(ET, 128, TB).transpose(1, 0, 2)).reshape(128, ET * TB))

    encT = _f16(
        _f32(encoder_out).transpose(2, 0, 1).reshape(KT, 128, S * B))

    hidT = _f16(np.ascontiguousarray(
        _f32(hidden).T.reshape(KT, 128, B).transpose(1, 0, 2)
    ).reshape(128, KT * B))

    attin_sb = _f16(_lhsT_rows(_f32(attin_w).T, KT, KT))       # [128, 64*128]
    attout_sb = _f16(_lhsT_rows(_f32(attout_w).T, 2 * KT, KT))  # [128,128*128]
    attin_bT = _f32(attin_b).reshape(KT, 128).T.copy()
    attout_bT = _f32(attout_b).reshape(KT, 128).T.copy()

    w_ih = _f32(w_ih)
    w_hh = _f32(w_hh)
    bsum = _f32(b_ih) + _f32(b_hh)

    ident = np.zeros((128, 128), np.float32)
    np.fill_diagonal(ident, 1.0)
    base = {
        "encT": encT, "hidT": hidT, "ident_bf": _bf(ident),
        "attin_w_sb": attin_sb, "attout_w_sb": attout_sb,
        "attin_bT": attin_bT, "attout_bT": attout_bT,
        "embT_in": embT_in,
    }

    in_maps = []
    for k in range(NC):
        rows = np.concatenate(
            [np.arange(g * D + k * 128, g * D + (k + 1) * 128)
             for g in GATE_ORDER])              # [512] i/f/o/g rows of core k
        wsel = w_ih[rows]
        wctx = wsel[:, :D]
        wemb = np.zeros((512, EP), np.float32)
        wemb[:, :E] = wsel[:, D:]
        whh = w_hh[rows]
        m = dict(base)
        m.update({
            "wctx_sb": _bf(_lhsT_rows(wctx.T, KT, 4)),   # [128, 32*128]
            "wemb_sb": _bf(_lhsT_rows(wemb.T, ET, 4)),   # [128, 12*128]
            "whh_sb": _bf(_lhsT_rows(whh.T, KT, 4)),     # [128, 32*128]
            "bias_kT": bsum[rows].reshape(4, 128).T.copy(),
            "gen_sb": _bf(np.ascontiguousarray(
                _f32(gen_w)[k * VS:(k + 1) * VS].T.reshape(KT, 128, VS)
                .transpose(1, 0, 2)).reshape(128, KT * VS)),
            "genb_bc": np.ascontiguousarray(np.broadcast_to(
                _f32(gen_b)[k * VS:(k + 1) * VS], (128, VS))),
        })
        in_maps.append(m)
    return in_maps


def build_kernel():
    nc = bacc.Bacc(None, num_devices=NC, target_bir_lowering=False,
                   debug=False)
    dp = nc.declare_dram_parameter
    encT = dp("encT", [KT, 128, S * B], F16, isOutput=False)
    hidT = dp("hidT", [128, KT * B], F16, isOutput=False)
    attin_w_d = dp("attin_w_sb", [128, KT * KT * 128], F16, isOutput=False)
    attout_w_d = dp("attout_w_sb", [128, 2 * KT * KT * 128], F16,
                    isOutput=False)
    attin_bT = dp("attin_bT", [128, KT], F32, isOutput=False)
    attout_bT = dp("attout_bT", [128, KT], F32, isOutput=False)
    embT_d = dp("embT_in", [128, ET * TB], BF16, isOutput=False)
    ident_d = dp("ident_bf", [128, 128], BF16, isOutput=False)
    wctx_d = dp("wctx_sb", [128, KT * 4 * 128], BF16, isOutput=False)
    wemb_d = dp("wemb_sb", [128, ET * 4 * 128], BF16, isOutput=False)
    whh_d = dp("whh_sb", [128, KT * 4 * 128], BF16, isOutput=False)
    bias_kT = dp("bias_kT", [128, 4], F32, isOutput=False)
    gen_d = dp("gen_sb", [128, KT * VS], BF16, isOutput=False)
    genb_d = dp("genb_bc", [128, VS], F32, isOutput=False)
    out = dp("out", [TB, VS], F32, isOutput=True)

    ag_outs = [
        nc.dram_tensor(f"ag{t}", [NC * 128, B], BF16, addr_space="Shared")
        for t in range(T)
    ]

    with tile.TileContext(nc) as tc:
        with (
            tc.tile_pool(name="live", bufs=1) as live,
        ):
            # ---- long-lived SBUF ----
            hsT = live.tile([128, KT, TB], BF16)          # gathered h.T
            xgT = live.tile([128, 4, TB], BF16)           # gate preacts
            ctxT = live.tile([128, KT * B], BF16)         # context.T
            cT = live.tile([128, B], F32)
            ident_sb = live.tile([128, 128], BF16)
            nc.sync.dma_start(ident_sb[:], ident_d[:])

            # =========== Phase A: attention (fp16 score path) ===========
            # bf16 absolute rounding of the large a/s_vec values (|a|~45,
            # |s|~400) swings unsaturated tanh preacts by O(1); fp16's
            # 11-bit significand keeps the final error ~4x smaller and
            # costs the same SBUF/matmul throughput.
            with (
                tc.tile_pool(name="pa", bufs=1) as pa,
                tc.tile_pool(name="psa", bufs=1, space="PSUM") as psa,
            ):
                hid_sb = pa.tile([128, KT * B], F16)
                nc.sync.dma_start(hid_sb[:], hidT[:])
                ainb_sb = pa.tile([128, KT], F32)
                nc.sync.dma_start(ainb_sb[:], attin_bT[:])
                aoutb_sb = pa.tile([128, KT], F32)
                nc.sync.dma_start(aoutb_sb[:], attout_bT[:])
                htT = pa.tile([128, KT * B], F16)
                ones32 = pa.tile([128, 1], F16)
                nc.vector.memset(ones32[:], 1.0)
                svb = pa.tile([128, KT * B], F16)
                a_bc = pa.tile([128, S * B], F16)

                with (
                    tc.tile_pool(name="pa1", bufs=1) as pa1,
                    tc.tile_pool(name="pa2", bufs=2) as pa2,
                ):
                    # full encoder in SBUF once: [128, KT, S*B] f16, 48KB/par
                    enc_all = pa1.tile([128, KT, S * B], F16)
                    nc.sync.dma_start(
                        enc_all[:], encT[:].rearrange("a p n -> p a n"))

                    with tc.tile_pool(name="pain", bufs=1) as pain:
                        attin_sb = pain.tile([128, KT * KT * 128], F16)
                        nc.scalar.dma_start(attin_sb[:], attin_w_d[:])
                        for j in range(KT):
                            ps_ht = psa.tile([128, B], F32, name="ps_ht",
                                             tag="ps_ht")
                            for kk in range(KT):
                                nc.tensor.matmul(
                                    ps_ht[:],
                                    attin_sb[:, (kk * KT + j) * 128:
                                             (kk * KT + j + 1) * 128],
                                    hid_sb[:, kk * B:(kk + 1) * B],
                                    start=(kk == 0), stop=(kk == KT - 1))
                            nc.scalar.activation(htT[:, j * B:(j + 1) * B],
                                                 ps_ht[:], AF.Identity,
                                                 bias=ainb_sb[:, j:j + 1])

                    # sc[s,b] = sum_d enc*ht -> PSUM [1, 3072] via ones-matmul
                    # products land exact in f32r (bf16 x bf16)
                    ps_sc = psa.tile([1, S * B], F32, name="ps_sc",
                                     tag="ps_sc")
                    for j in range(KT):
                        prod = pa2.tile([128, S * B], F16, name="prod",
                                        tag="prod")
                        nc.vector.tensor_tensor(
                            prod[:].rearrange("p (s b) -> p s b", s=S),
                            enc_all[:, j].rearrange("p (s b) -> p s b", s=S),
                            htT[:, j * B:(j + 1) * B].unsqueeze(1)
                               .broadcast_to([128, S, B]),
                            ALU.mult)
                        for c in range(6):
                            nc.tensor.matmul(
                                ps_sc[:, c * 512:(c + 1) * 512],
                                ones32[:],
                                prod[:, c * 512:(c + 1) * 512],
                                start=(j == 0), stop=(j == KT - 1))

                    # log_softmax over b, on partition-0 row [1, 3072]
                    sc_row = pa1.tile([1, S * B], F32)
                    nc.vector.tensor_copy(sc_row[:], ps_sc[:])
                    scv = sc_row[:].rearrange("o (s b) -> o s b", s=S)
                    mx = pa1.tile([1, S], F32)
                    nc.vector.tensor_reduce(mx[:], scv, mybir.AxisListType.X,
                                            ALU.max)
                    mxb = mx[:].unsqueeze(2).broadcast_to([1, S, B])
                    exr = pa1.tile([1, S * B], F32)
                    nc.vector.tensor_tensor(
                        exr[:].rearrange("o (s b) -> o s b", s=S), scv, mxb,
                        ALU.subtract)
                    nc.scalar.activation(exr[:], exr[:], AF.Exp)
                    sume = pa1.tile([1, S], F32)
                    nc.vector.tensor_reduce(
                        sume[:], exr[:].rearrange("o (s b) -> o s b", s=S),
                        mybir.AxisListType.X, ALU.add)
                    lnz = pa1.tile([1, S], F32)
                    nc.scalar.activation(lnz[:], sume[:], AF.Ln)
                    tot = pa1.tile([1, S], F32)
                    nc.vector.tensor_tensor(tot[:], lnz[:], mx[:], ALU.add)
                    a_row = pa1.tile([1, S * B], F16)
                    nc.vector.tensor_tensor(
                        a_row[:].rearrange("o (s b) -> o s b", s=S), scv,
                        tot[:].unsqueeze(2).broadcast_to([1, S, B]),
                        ALU.subtract)
                    # broadcast a_row to all partitions via K=1 ones-matmul
                    ones128 = pa1.tile([1, 128], F16)
                    nc.vector.memset(ones128[:], 1.0)
                    for c in range(6):
                        ps_ab = psa.tile([128, 512], F32, name="ps_ab",
                                         tag="ps_ab")
                        nc.tensor.matmul(ps_ab[:], ones128[:],
                                         a_row[0:1, c * 512:(c + 1) * 512],
                                         start=True, stop=True)
                        nc.vector.tensor_copy(a_bc[:, c * 512:(c + 1) * 512],
                                              ps_ab[:])

                    # s_vec.T [128, 8*48]: contiguous halving-tree over s
                    # (strided X-reduce runs ~5x slower on DVE)
                    for j in range(KT):
                        prod2 = pa2.tile([128, S, B], F16, name="prod2",
                                         tag="prod")
                        nc.vector.tensor_tensor(
                            prod2[:].rearrange("p s b -> p (s b)"),
                            enc_all[:, j], a_bc[:], ALU.mult)
                        w = S // 2
                        while w >= 1:
                            dst = (prod2[:, 0:w, :] if w > 1
                                   else svb[:, j * B:(j + 1) * B]
                                   .rearrange("p (s b) -> p s b", s=1))
                            nc.vector.tensor_tensor(
                                dst, prod2[:, 0:w, :], prod2[:, w:2 * w, :],
                                ALU.add)
                            w //= 2

                with tc.tile_pool(name="pa3", bufs=1) as pa3:
                    attout_sb = pa3.tile([128, 2 * KT * KT * 128], F16)
                    nc.sync.dma_start(attout_sb[:], attout_w_d[:])
                    for m in range(KT):
                        ps_ctx = psa.tile([128, B], F32, name="ps_ctx",
                                          tag="ps_ht")
                        for kk in range(2 * KT):
                            rhs = (svb[:, kk * B:(kk + 1) * B] if kk < KT
                                   else hid_sb[:, (kk - KT) * B:
                                               (kk - KT + 1) * B])
                            nc.tensor.matmul(
                                ps_ctx[:],
                                attout_sb[:, (kk * KT + m) * 128:
                                          (kk * KT + m + 1) * 128],
                                rhs, start=(kk == 0), stop=(kk == 2 * KT - 1))
                        nc.scalar.activation(
                            ctxT[:, m * B:(m + 1) * B], ps_ctx[:], AF.Tanh,
                            bias=aoutb_sb[:, m:m + 1])

            # =========== Phase B: embedding + gate preacts ===========
            with (
                tc.tile_pool(name="pb", bufs=1) as pb,
                tc.tile_pool(name="psb", bufs=2, space="PSUM") as psb,
            ):
                embT = pb.tile([128, ET, TB], BF16)
                nc.sync.dma_start(
                    embT[:].rearrange("p a n -> p (a n)"), embT_d[:])

                wctx_sb = pb.tile([128, KT * 4 * 128], BF16)
                nc.sync.dma_start(wctx_sb[:], wctx_d[:])
                wemb_sb = pb.tile([128, ET * 4 * 128], BF16)
                nc.sync.dma_start(wemb_sb[:], wemb_d[:])
                bias_sb = pb.tile([128, 4], F32)
                nc.sync.dma_start(bias_sb[:], bias_kT[:])

                # ctx gate contribution (constant over t) + bias
                ctxg = pb.tile([128, 4, B], F32)
                for g in range(4):
                    ps_cg = psb.tile([128, B], F32, name="ps_cg", tag="ps_cg")
                    for kk in range(KT):
                        nc.tensor.matmul(
                            ps_cg[:],
                            wctx_sb[:, (kk * 4 + g) * 128:
                                    (kk * 4 + g + 1) * 128],
                            ctxT[:, kk * B:(kk + 1) * B],
                            start=(kk == 0), stop=(kk == KT - 1))
                    nc.scalar.activation(ctxg[:, g, :], ps_cg[:], AF.Identity,
                                         bias=bias_sb[:, g:g + 1])

                # xgT = emb part + ctxg broadcast; chunks of 8 timesteps
                for g in range(4):
                    for c in range(8):
                        ps_xg = psb.tile([128, 8 * B], F32, name="ps_xg",
                                         tag="ps_xg")
                        for kk in range(ET):
                            nc.tensor.matmul(
                                ps_xg[:],
                                wemb_sb[:, (kk * 4 + g) * 128:
                                        (kk * 4 + g + 1) * 128],
                                embT[:, kk, c * 8 * B:(c + 1) * 8 * B],
                                start=(kk == 0), stop=(kk == ET - 1))
                        nc.vector.tensor_tensor(
                            xgT[:, g, c * 8 * B:(c + 1) * 8 * B]
                            .rearrange("p (t b) -> p t b", b=B),
                            ps_xg[:].rearrange("p (t b) -> p t b", b=B),
                            ctxg[:, g, :].unsqueeze(1)
                                         .broadcast_to([128, 8, B]),
                            ALU.add)

            # =========== Phase C: scan + vocab projection ===========
            with (
                tc.tile_pool(name="pc", bufs=1) as pc,
                tc.tile_pool(name="pc3", bufs=3) as pc3,
                tc.tile_pool(name="pdram", bufs=3, space="DRAM") as pdram,
                tc.tile_pool(name="psc", bufs=2, space="PSUM") as psc,
                tc.tile_pool(name="psf", bufs=4, space="PSUM") as psf,
            ):
                gen_sb = pc.tile([128, KT * VS], BF16)
                nc.sync.dma_start(gen_sb[:], gen_d[:])
                whh_sb = pc.tile([128, KT * 4 * 128], BF16)
                nc.sync.dma_start(whh_sb[:], whh_d[:])
                genb_sb = pc.tile([128, VS], F32)
                nc.sync.dma_start(genb_sb[:], genb_d[:])

                def emit_chunk(m, n):
                    ps_f = psf.tile([128, VCH], F32, name="ps_f", tag="ps_f")
                    for kk in range(KT):
                        nc.tensor.matmul(
                            ps_f[:],
                            hsT[:, kk, m * 128:(m + 1) * 128],
                            gen_sb[:, kk * VS + n * VCH:
                                   kk * VS + (n + 1) * VCH],
                            start=(kk == 0), stop=(kk == KT - 1))
                    o_sb = pc3.tile([128, VCH], F32, name="o_sb", tag="o_sb")
                    nc.vector.tensor_tensor(
                        o_sb[:], ps_f[:],
                        genb_sb[:, n * VCH:(n + 1) * VCH], ALU.add)
                    nc.gpsimd.dma_start(
                        out[m * 128:(m + 1) * 128,
                            n * VCH:(n + 1) * VCH], o_sb[:])

                next_m = 0
                next_n = 0
                for t in range(T):
                    # --- vocab chunks FIRST in program order: their matmuls
                    # have no dependency on this step's AllGather, so the
                    # in-order TensorE queue streams them during the previous
                    # step's exchange window. Only rows gathered in prior
                    # steps are eligible.
                    avail_m = (t * B) // 128
                    emitted = 0
                    while emitted < 4 and next_m < avail_m:
                        emit_chunk(next_m, next_n)
                        next_n += 1
                        emitted += 1
                        if next_n == NCH:
                            next_n = 0
                            next_m += 1

                    # --- gates for step t (one psum bank, 4 gate groups) ---
                    ps = psc.tile([128, 4, B], F32, name="ps_g", tag="ps_g")
                    nc.tensor.matmul(
                        ps[:, :, :], ident_sb[:],
                        xgT[:, :, t * B:(t + 1) * B],
                        start=True, stop=(t == 0))
                    if t > 0:
                        for kk in range(KT):
                            for g in range(4):
                                nc.tensor.matmul(
                                    ps[:, g, :],
                                    whh_sb[:, (kk * 4 + g) * 128:
                                           (kk * 4 + g + 1) * 128],
                                    hsT[:, kk, (t - 1) * B:t * B],
                                    start=False, stop=(kk == KT - 1))

                    # --- activations: sigmoid(i,f,o) fused, tanh(g) ---
                    sio = pc3.tile([128, 3, B], F32, name="sio", tag="sio")
                    nc.scalar.activation(sio[:, :, :], ps[:, 0:3, :],
                                         AF.Sigmoid)
                    tg = pc3.tile([128, B], F32, name="tg", tag="tg")
                    nc.scalar.activation(tg[:], ps[:, 3, :], AF.Tanh)

                    if t == 0:
                        nc.vector.tensor_tensor(cT[:], sio[:, 0, :], tg[:],
                                                ALU.mult)
                    else:
                        m2 = pc3.tile([128, B], F32, name="m2", tag="m2")
                        nc.vector.tensor_tensor(m2[:], sio[:, 0, :], tg[:],
                                                ALU.mult)
                        m1 = pc3.tile([128, B], F32, name="m1", tag="m1")
                        nc.vector.tensor_tensor(m1[:], sio[:, 1, :], cT[:],
                                                ALU.mult)
                        nc.vector.tensor_tensor(cT[:], m1[:], m2[:], ALU.add)
                    tc_ = pc3.tile([128, B], F32, name="tc_", tag="tc_")
                    nc.scalar.activation(tc_[:], cT[:], AF.Tanh)
                    h_bf = pc3.tile([128, B], BF16, name="h_bf", tag="h_bf")
                    nc.vector.tensor_tensor(h_bf[:], sio[:, 2, :], tc_[:],
                                            ALU.mult)

                    hb_d = pdram.tile([128, B], BF16, name="hb_d", tag="hb_d")
                    nc.sync.dma_start(hb_d[:], h_bf[:])
                    nc.gpsimd.collective_compute(
                        "AllGather", ALU.bypass,
                        replica_groups=[list(range(NC))],
                        ins=[hb_d.opt()],
                        outs=[ag_outs[t][:]],
                    )
                    nc.sync.dma_start(
                        hsT[:, :, t * B:(t + 1) * B],
                        ag_outs[t][:].rearrange("(j p) b -> p j b", p=128))
                while next_m < MT:
                    emit_chunk(next_m, next_n)
                    next_n += 1
                    if next_n == NCH:
                        next_n = 0
                        next_m += 1
    nc.compile()
    return nc


_NC_CACHE = {}


def get_nc():
    if "nc" not in _NC_CACHE:
        _NC_CACHE["nc"] = build_kernel()
    return _NC_CACHE["nc"]


def kernel(**inputs):
    in_maps = prep_in_maps(**inputs)
    nc = get_nc()
    res = run_bass_kernel_spmd(nc, in_maps, core_ids=list(range(NC)))
    shards = [np.asarray(res.results[k]["out"]).reshape(T, B, VS)
              for k in range(NC)]
    return np.concatenate(shards, axis=2)


if __name__ == "__main__":
    print("building...")
    get_nc()
    print("built ok")
